# revision 14
# baseline (speedup 1.0000x reference)
"""Causal self-attention (B=4, T=2048, C=1024, H=16, D=64) on 8 trn2 NeuronCores.

Sharding: tensor-parallel over heads. Each core owns 2 heads (128 channels):
  - computes Q^T/K^T (transposed) and natural V (via PE transpose) for its
    heads from the host-pretransposed full x^T,
  - causal attention with S^T strips (keys on partitions) + exp + diag mask,
  - AV in TRANSPOSED, weight-stationary form: for each key-block c the
    V-block (augmented with a ones column) stays loaded in the PE array and
    the exp'd score strips stream through 512 columns at a time,
    accumulating attOut^T [65, T] in PSUM (row 64 = softmax denominator),
  - normalization: reciprocal of the denominator row, rank-1 broadcast
    matmul (ones x recip-row) to all 64 partitions, then one elementwise
    multiply per 512-col quarter,
  - partial output projection with its 128 rows of W_proj (bf16 out).
Host sums the 8 partial projections and adds (b_v @ W_proj + b_proj).

The per-core program is identical (SPMD); only the weight-slice inputs differ.
"""

import numpy as np
import ml_dtypes

import concourse.bass as bass
import concourse.bacc as bacc
import concourse.mybir as mybir
import concourse.tile as tile

B, T, C, H, D = 4, 2048, 1024, 16, 64
NCORES = 8
HPC = H // NCORES  # heads per core = 2
P = 128
NB = T // P  # 16 key/query blocks of 128
CK = C // P  # 8 contraction chunks for the projections
NQ = T // 512  # 4 query "quarters" of 512

F32 = mybir.dt.float32
F32R = mybir.dt.float32r
BF16 = mybir.dt.bfloat16
MULT = mybir.AluOpType.mult
EXP = mybir.ActivationFunctionType.Exp

# at_sb layout: one contiguous causal run per key-block c covering
# q-columns [128c, T). OFFC[c] = start offset of run c.
OFFC = [c * T - 64 * c * (c - 1) for c in range(NB)]
AT_W = OFFC[NB - 1] + (T - 128 * (NB - 1))  # 17408


def attention_body(tc, outs, ins):
    nc = tc.nc
    xt = ins["xt"]          # [C, B*T] bf16 (x transposed, col = b*T + t)
    wq = ins["wq"]          # [C, 128] bf16
    wk = ins["wk"]          # [C, 128] bf16
    wv = ins["wv"]          # [C, 128] bf16
    wp = ins["wp"]          # [128, C] bf16
    bq = ins["bq"]          # [128, 1] f32 (prescaled by 1/sqrt(D))
    bk = ins["bk"]          # [128, 1] f32
    maskt = ins["maskt"]    # [128, 128] bf16: 1 if k<=q else 0
    ident = ins["ident"]    # [128, 128] bf16 identity
    out = outs["out"]       # [B*T, C] bf16 partial projection output

    with (
        tc.tile_pool(name="consts", bufs=1) as consts,
        tc.tile_pool(name="xtp", bufs=3) as xtp,
        tc.tile_pool(name="qkp", bufs=2) as qkp,
        tc.tile_pool(name="vp", bufs=2) as vp,
        tc.tile_pool(name="atp", bufs=2) as atp,
        tc.tile_pool(name="attnp", bufs=2) as attnp,
        tc.tile_pool(name="smallp", bufs=4) as smallp,
        tc.tile_pool(name="outp", bufs=3) as outp,
        tc.tile_pool(name="pp", bufs=4, space="PSUM") as pp,
    ):
        # ---- constants ----
        wq_sb = consts.tile([P, CK, P], BF16, name="wq_sb")
        nc.sync.dma_start(wq_sb, wq.rearrange("(o p) m -> p o m", p=P))
        wk_sb = consts.tile([P, CK, P], BF16, name="wk_sb")
        nc.sync.dma_start(wk_sb, wk.rearrange("(o p) m -> p o m", p=P))
        wv_sb = consts.tile([P, CK, P], BF16, name="wv_sb")
        nc.sync.dma_start(wv_sb, wv.rearrange("(o p) m -> p o m", p=P))
        wp_bf = consts.tile([P, C], BF16, name="wp_bf")
        nc.sync.dma_start(wp_bf, wp)
        bq_sb = consts.tile([P, 1], F32, name="bq_sb")
        nc.gpsimd.dma_start(bq_sb, bq)
        bk_sb = consts.tile([P, 1], F32, name="bk_sb")
        nc.gpsimd.dma_start(bk_sb, bk)
        mask_sb = consts.tile([P, P], BF16, name="mask_sb")
        nc.gpsimd.dma_start(mask_sb, maskt)
        id_sb = consts.tile([P, P], BF16, name="id_sb")
        nc.gpsimd.dma_start(id_sb, ident)
        ones_c = consts.tile([P, D], BF16, name="ones_c")
        nc.vector.memset(ones_c, 1.0)

        def qkv_phase(b):
            """Q^T, K^T (bf16, scaled/biased) + natural V augmented with a
            ones column. Returns (qt, kt, vaug)."""
            qt = qkp.tile([P, T], BF16, tag="qt", name=f"qt_{b}")
            kt = qkp.tile([P, T], BF16, tag="kt", name=f"kt_{b}")
            vaug = vp.tile([P, NB, HPC, D + 1], BF16, tag="vaug", name=f"vaug_{b}")
            nc.gpsimd.memset(vaug[:, :, :, D:], 1.0)  # softmax-denominator col

            for q4 in range(NQ):
                lo = q4 * 512
                xq = xtp.tile([P, CK, 512], BF16, tag="xq", name=f"xq_{b}_{q4}")
                nc.sync.dma_start(
                    xq, xt[:, b * T + lo : b * T + lo + 512].rearrange("(o p) t -> p o t", p=P)
                )
                ps_q = pp.tile([P, 512], F32, tag="mm", bufs=2, name=f"psq_{b}_{q4}")
                for cc in range(CK):
                    nc.tensor.matmul(
                        ps_q, lhsT=wq_sb[:, cc], rhs=xq[:, cc],
                        start=(cc == 0), stop=(cc == CK - 1),
                    )
                nc.vector.tensor_scalar(
                    qt[:, lo : lo + 512], ps_q, 0.125, bq_sb,
                    mybir.AluOpType.mult, mybir.AluOpType.add,
                )
                ps_k = pp.tile([P, 512], F32, tag="mm", bufs=2, name=f"psk_{b}_{q4}")
                for cc in range(CK):
                    nc.tensor.matmul(
                        ps_k, lhsT=wk_sb[:, cc], rhs=xq[:, cc],
                        start=(cc == 0), stop=(cc == CK - 1),
                    )
                nc.vector.tensor_scalar(
                    kt[:, lo : lo + 512], ps_k, bk_sb, None, mybir.AluOpType.add,
                )
                # V^T quarter -> cast bf16 -> transpose to natural V chunks
                ps_v = pp.tile([P, 512], F32, tag="mm", bufs=2, name=f"psv_{b}_{q4}")
                for cc in range(CK):
                    nc.tensor.matmul(
                        ps_v, lhsT=wv_sb[:, cc], rhs=xq[:, cc],
                        start=(cc == 0), stop=(cc == CK - 1),
                    )
                vt = vp.tile([P, 512], BF16, tag="vt", bufs=2, name=f"vt_{b}_{q4}")
                nc.vector.tensor_copy(vt, ps_v)
                # full 128x128 transposes cover both heads per token-block
                vtp = pp.tile([P, 4, P], BF16, tag="mm", bufs=2, name=f"vtp_{b}_{q4}")
                for t4 in range(4):
                    nc.tensor.matmul(
                        vtp[:, t4],
                        lhsT=vt[:, t4 * P : (t4 + 1) * P],
                        rhs=id_sb,
                        is_transpose=True,
                        start=True, stop=True,
                    )
                nc.vector.tensor_copy(
                    vaug[:, q4 * 4 : (q4 + 1) * 4, :, 0:D],
                    vtp.rearrange("p a (h d) -> p a h d", h=HPC),
                )
            return qt, kt, vaug

        def s_chunks(b, h, qt, kt, at):
            """Generator: S^T causal strips + exp + diagonal mask into at.
            Yields after each chunk so callers can interleave other PE work."""
            hs = h * D
            for c in range(NB):
                w = T - 128 * c
                lhs_k = kt[hs : hs + D, c * P : (c + 1) * P]
                for ck in range(0, w, 512):
                    n = min(512, w - ck)
                    sps = pp.tile([P, 512], F32, tag="wk", bufs=4, name=f"sps_{b}_{h}_{c}_{ck}")
                    nc.tensor.matmul(
                        sps[:, :n], lhsT=lhs_k,
                        rhs=qt[hs : hs + D, 128 * c + ck : 128 * c + ck + n],
                        start=True, stop=True,
                    )
                    o = OFFC[c] + ck
                    nc.scalar.activation(at[:, o : o + n], sps[:, :n], EXP)
                    if ck == 0:  # diagonal block: multiplicative causal mask
                        nc.gpsimd.tensor_tensor(
                            at[:, o : o + P], at[:, o : o + P], mask_sb, MULT
                        )
                    yield

        def avt_phase(b, h, at, vaug, attn_sb, attn1):
            """Weight-stationary transposed AV: attOut^T [65, T] in PSUM
            (row 64 = denominator), then normalize into attn tiles."""
            avp = [
                pp.tile([P, 512], F32, tag="wk", bufs=4, name=f"avp_{b}_{h}_{g}")
                for g in range(NQ)
            ]
            for c in range(NB):
                g_min = max(0, (c - 3 + 3) // 4)  # smallest g with 4g+3 >= c
                for g in range(g_min, NQ):
                    if c <= 4 * g:
                        rhs = at[:, OFFC[c] + 512 * g - 128 * c : OFFC[c] + 512 * g - 128 * c + 512]
                        o1 = 0
                    else:
                        wdt = 512 - 128 * (c - 4 * g)
                        rhs = at[:, OFFC[c] : OFFC[c] + wdt]
                        o1 = 512 - wdt
                    nc.tensor.matmul(
                        avp[g][0 : D + 1, o1:512],
                        lhsT=vaug[:, c, h],
                        rhs=rhs,
                        start=(c == 0), stop=(c == 4 * g + 3),
                        skip_group_check=True,
                    )
            for g in range(NQ):
                # denominator row psum->sbuf (stays on partition 64)
                dsb = smallp.tile([D + 1, 512], BF16, tag="dsb", bufs=2, name=f"dsb_{b}_{h}_{g}")
                nc.vector.tensor_copy(dsb[D : D + 1, :], avp[g][D : D + 1, :])
                # rank-1 broadcast of the denominator to partitions 0..63
                rbc = pp.tile([D, 512], F32, tag="rbc", bufs=2, name=f"rbc_{b}_{h}_{g}")
                nc.tensor.matmul(
                    rbc,
                    lhsT=ones_c[D : D + 1, :],
                    rhs=dsb[D : D + 1, :],
                    start=True, stop=True,
                )
                rbc_sb = smallp.tile([D, 512], F32, tag="rbc_sb", bufs=2, name=f"rbcs_{b}_{h}_{g}")
                nc.vector.reciprocal_approx_fast(out=rbc_sb, in_=rbc)
                dst = attn_sb[0:D, g * 512 : (g + 1) * 512] if h == 0 else attn1[:, g * 512 : (g + 1) * 512]
                nc.vector.tensor_tensor(dst, avp[g][0:D, :], rbc_sb, MULT)
                if h == 1:  # shift head-1 rows to partitions 64..127
                    nc.sync.dma_start(
                        attn_sb[D : 2 * D, g * 512 : (g + 1) * 512],
                        attn1[:, g * 512 : (g + 1) * 512],
                    )

        def proj_block(b, attn_sb, j):
            outst = outp.tile([P, C], BF16, tag="outst", name=f"outst_{b}_{j}")
            for n2 in range(2):
                pps = pp.tile([P, 512], F32, tag="mm", bufs=2, name=f"pps_{b}_{j}_{n2}")
                nc.tensor.matmul(
                    pps,
                    lhsT=attn_sb[:, j * P : (j + 1) * P],
                    rhs=wp_bf[:, n2 * 512 : (n2 + 1) * 512],
                    start=True, stop=True,
                )
                nc.vector.tensor_copy(outst[:, n2 * 512 : (n2 + 1) * 512], pps)
            nc.sync.dma_start(out[b * T + j * P : b * T + (j + 1) * P, :], outst)

        # ---- software pipeline over batches ----
        qt, kt, vaug = qkv_phase(0)
        ats = [atp.tile([P, AT_W], BF16, tag="at", name=f"at_0_{h}") for h in range(HPC)]
        for h in range(HPC):
            for _ in s_chunks(0, h, qt, kt, ats[h]):
                pass
        for b in range(B):
            if b + 1 < B:
                qt_n, kt_n, vaug_n = qkv_phase(b + 1)
            attn_sb = attnp.tile([P, T], BF16, tag="attn", name=f"attn_{b}")
            attn1 = attnp.tile([D, T], BF16, tag="attn1", name=f"attn1_{b}")
            for h in range(HPC):
                avt_phase(b, h, ats[h], vaug, attn_sb, attn1)
            js = list(range(NB))
            if b + 1 < B:
                # interleave proj(b) blocks into the exp-bound S(b+1) phase
                ats = [atp.tile([P, AT_W], BF16, tag="at", name=f"at_{b+1}_{h}") for h in range(HPC)]
                nchunk = 0
                for h in range(HPC):
                    for _ in s_chunks(b + 1, h, qt_n, kt_n, ats[h]):
                        nchunk += 1
                        if nchunk >= 12 and (nchunk - 12) % 4 == 0 and js:
                            proj_block(b, attn_sb, js.pop(0))
                qt, kt, vaug = qt_n, kt_n, vaug_n
            for j in js:
                proj_block(b, attn_sb, j)


def build_nc():
    nc = bacc.Bacc("TRN2", debug=False, enable_asserts=False, num_devices=NCORES)
    ins = {
        "xt": nc.dram_tensor("xt", [C, B * T], BF16, kind="ExternalInput").ap(),
        "wq": nc.dram_tensor("wq", [C, P], BF16, kind="ExternalInput").ap(),
        "wk": nc.dram_tensor("wk", [C, P], BF16, kind="ExternalInput").ap(),
        "wv": nc.dram_tensor("wv", [C, P], BF16, kind="ExternalInput").ap(),
        "wp": nc.dram_tensor("wp", [P, C], BF16, kind="ExternalInput").ap(),
        "bq": nc.dram_tensor("bq", [P, 1], F32, kind="ExternalInput").ap(),
        "bk": nc.dram_tensor("bk", [P, 1], F32, kind="ExternalInput").ap(),
        "maskt": nc.dram_tensor("maskt", [P, P], BF16, kind="ExternalInput").ap(),
        "ident": nc.dram_tensor("ident", [P, P], BF16, kind="ExternalInput").ap(),
    }
    outs = {"out": nc.dram_tensor("out", [B * T, C], BF16, kind="ExternalOutput").ap()}
    with tile.TileContext(nc) as tc:
        attention_body(tc, outs, ins)
    nc.compile()
    return nc


def make_in_maps(inputs, W_qkv, b_qkv, W_proj):
    x2 = np.asarray(inputs, np.float32).reshape(B * T, C)
    xtv = np.ascontiguousarray(x2.T).astype(ml_dtypes.bfloat16)
    W_qkv = np.asarray(W_qkv, np.float32)
    b_qkv = np.asarray(b_qkv, np.float32)
    W_proj = np.asarray(W_proj, np.float32)
    identv = np.eye(P, dtype=ml_dtypes.bfloat16)
    masktv = np.triu(np.ones((P, P), np.float32)).astype(ml_dtypes.bfloat16)
    in_maps = []
    for cid in range(NCORES):
        s = cid * HPC * D
        in_maps.append({
            "xt": xtv,
            "wq": np.ascontiguousarray(W_qkv[:, s : s + P]).astype(ml_dtypes.bfloat16),
            "wk": np.ascontiguousarray(W_qkv[:, C + s : C + s + P]).astype(ml_dtypes.bfloat16),
            "wv": np.ascontiguousarray(W_qkv[:, 2 * C + s : 2 * C + s + P]).astype(ml_dtypes.bfloat16),
            "wp": np.ascontiguousarray(W_proj[s : s + P, :]).astype(ml_dtypes.bfloat16),
            "bq": np.ascontiguousarray(b_qkv[s : s + P].reshape(P, 1) * 0.125),
            "bk": np.ascontiguousarray(b_qkv[C + s : C + s + P].reshape(P, 1)),
            "maskt": masktv,
            "ident": identv,
        })
    return in_maps


_NC_CACHE = {}


def run(inputs, W_qkv, b_qkv, W_proj, b_proj, trace=False, **kw):
    from concourse.bass_utils import run_bass_kernel_spmd

    if "nc" not in _NC_CACHE:
        _NC_CACHE["nc"] = build_nc()
    nc = _NC_CACHE["nc"]
    in_maps = make_in_maps(inputs, W_qkv, b_qkv, W_proj)
    res = run_bass_kernel_spmd(nc, in_maps, core_ids=list(range(NCORES)), trace=trace, **kw)
    acc = res.results[0]["out"].astype(np.float32)
    for cid in range(1, NCORES):
        acc += res.results[cid]["out"].astype(np.float32)
    host_bias = np.asarray(b_qkv, np.float32)[2 * C :] @ np.asarray(W_proj, np.float32)
    host_bias = host_bias + np.asarray(b_proj, np.float32)
    outv = (acc + host_bias[None, :]).reshape(B, T, C).astype(np.float32)
    return outv, res


def kernel(inputs, W_qkv, b_qkv, W_proj, b_proj):
    outv, _ = run(inputs, W_qkv, b_qkv, W_proj, b_proj, trace=False)
    return outv


# revision 17
# speedup vs baseline: 1.0707x; 1.0707x over previous
"""Causal self-attention (B=4, T=2048, C=1024, H=16, D=64) on 8 trn2 NeuronCores.

Sharding: tensor-parallel over heads. Each core owns 2 heads (128 channels):
  - computes Q^T/K^T (transposed) and natural V (via PE transpose) for its
    heads from the host-pretransposed full x^T,
  - causal attention with S^T strips (keys on partitions) + exp + diag mask,
  - AV in TRANSPOSED, weight-stationary form: for each key-block c the
    V-block (augmented with a ones column) stays loaded in the PE array and
    the exp'd score strips stream through 512 columns at a time,
    accumulating attOut^T [65, T] in PSUM (row 64 = softmax denominator),
  - normalization: reciprocal of the denominator row, rank-1 broadcast
    matmul (ones x recip-row) to all 64 partitions, then one elementwise
    multiply per 512-col quarter,
  - partial output projection with its 128 rows of W_proj (bf16 out).
Host sums the 8 partial projections and adds (b_v @ W_proj + b_proj).

The per-core program is identical (SPMD); only the weight-slice inputs differ.
"""

import numpy as np
import ml_dtypes

import concourse.bass as bass
import concourse.bacc as bacc
import concourse.mybir as mybir
import concourse.tile as tile

B, T, C, H, D = 4, 2048, 1024, 16, 64
NCORES = 8
HPC = H // NCORES  # heads per core = 2
P = 128
NB = T // P  # 16 key/query blocks of 128
CK = C // P  # 8 contraction chunks for the projections
NQ = T // 512  # 4 query "quarters" of 512

F32 = mybir.dt.float32
F32R = mybir.dt.float32r
BF16 = mybir.dt.bfloat16
MULT = mybir.AluOpType.mult
EXP = mybir.ActivationFunctionType.Exp

# at_sb layout: one contiguous causal run per key-block c covering
# q-columns [128c, T). OFFC[c] = start offset of run c.
OFFC = [c * T - 64 * c * (c - 1) for c in range(NB)]
AT_W = OFFC[NB - 1] + (T - 128 * (NB - 1))  # 17408


def attention_body(tc, outs, ins):
    nc = tc.nc
    xt = ins["xt"]          # [C, B*T] bf16 (x transposed, col = b*T + t)
    wq = ins["wq"]          # [C, 128] bf16
    wk = ins["wk"]          # [C, 128] bf16
    wv = ins["wv"]          # [C, 128] bf16
    wp = ins["wp"]          # [128, C] bf16
    bq = ins["bq"]          # [128, 1] f32 (prescaled by 1/sqrt(D))
    bk = ins["bk"]          # [128, 1] f32
    maskt = ins["maskt"]    # [128, 128] bf16: 1 if k<=q else 0
    ident = ins["ident"]    # [128, 128] bf16 identity
    out = outs["out"]       # [B*T, C] bf16 partial projection output

    with (
        tc.tile_pool(name="consts", bufs=1) as consts,
        tc.tile_pool(name="xtp", bufs=3) as xtp,
        tc.tile_pool(name="qkp", bufs=2) as qkp,
        tc.tile_pool(name="vp", bufs=2) as vp,
        tc.tile_pool(name="atp", bufs=2) as atp,
        tc.tile_pool(name="attnp", bufs=2) as attnp,
        tc.tile_pool(name="smallp", bufs=4) as smallp,
        tc.tile_pool(name="outp", bufs=3) as outp,
        tc.tile_pool(name="pp", bufs=4, space="PSUM") as pp,
    ):
        # ---- constants ----
        wq_sb = consts.tile([P, CK, P], BF16, name="wq_sb")
        nc.sync.dma_start(wq_sb, wq.rearrange("(o p) m -> p o m", p=P))
        wk_sb = consts.tile([P, CK, P], BF16, name="wk_sb")
        nc.sync.dma_start(wk_sb, wk.rearrange("(o p) m -> p o m", p=P))
        wv_sb = consts.tile([P, CK, P], BF16, name="wv_sb")
        nc.sync.dma_start(wv_sb, wv.rearrange("(o p) m -> p o m", p=P))
        wp_bf = consts.tile([P, C], BF16, name="wp_bf")
        nc.sync.dma_start(wp_bf, wp)
        bq_sb = consts.tile([P, 1], F32, name="bq_sb")
        nc.gpsimd.dma_start(bq_sb, bq)
        bk_sb = consts.tile([P, 1], F32, name="bk_sb")
        nc.gpsimd.dma_start(bk_sb, bk)
        mask_sb = consts.tile([P, P], BF16, name="mask_sb")
        nc.gpsimd.dma_start(mask_sb, maskt)
        id_sb = consts.tile([P, P], BF16, name="id_sb")
        nc.gpsimd.dma_start(id_sb, ident)
        ones_c = consts.tile([P, D], BF16, name="ones_c")
        nc.vector.memset(ones_c, 1.0)

        def qkv_phase(b):
            """Q^T, K^T (bf16, scaled/biased) + natural V augmented with a
            ones column. Returns (qt, kt, vaug)."""
            qt = qkp.tile([P, T], BF16, tag="qt", name=f"qt_{b}")
            kt = qkp.tile([P, T], BF16, tag="kt", name=f"kt_{b}")
            vaug = vp.tile([P, NB, HPC, D + 1], BF16, tag="vaug", name=f"vaug_{b}")
            nc.gpsimd.memset(vaug[:, :, :, D:], 1.0)  # softmax-denominator col

            for q4 in range(NQ):
                lo = q4 * 512
                xq = xtp.tile([P, CK, 512], BF16, tag="xq", name=f"xq_{b}_{q4}")
                nc.sync.dma_start(
                    xq, xt[:, b * T + lo : b * T + lo + 512].rearrange("(o p) t -> p o t", p=P)
                )
                ps_q = pp.tile([P, 512], F32, tag="mm", bufs=2, name=f"psq_{b}_{q4}")
                for cc in range(CK):
                    nc.tensor.matmul(
                        ps_q, lhsT=wq_sb[:, cc], rhs=xq[:, cc],
                        start=(cc == 0), stop=(cc == CK - 1),
                    )
                nc.vector.tensor_scalar(
                    qt[:, lo : lo + 512], ps_q, 0.125, bq_sb,
                    mybir.AluOpType.mult, mybir.AluOpType.add,
                )
                ps_k = pp.tile([P, 512], F32, tag="mm", bufs=2, name=f"psk_{b}_{q4}")
                for cc in range(CK):
                    nc.tensor.matmul(
                        ps_k, lhsT=wk_sb[:, cc], rhs=xq[:, cc],
                        start=(cc == 0), stop=(cc == CK - 1),
                    )
                nc.vector.tensor_scalar(
                    kt[:, lo : lo + 512], ps_k, bk_sb, None, mybir.AluOpType.add,
                )
                # V^T quarter -> cast bf16 -> transpose to natural V chunks
                ps_v = pp.tile([P, 512], F32, tag="mm", bufs=2, name=f"psv_{b}_{q4}")
                for cc in range(CK):
                    nc.tensor.matmul(
                        ps_v, lhsT=wv_sb[:, cc], rhs=xq[:, cc],
                        start=(cc == 0), stop=(cc == CK - 1),
                    )
                vt = vp.tile([P, 512], BF16, tag="vt", bufs=2, name=f"vt_{b}_{q4}")
                nc.vector.tensor_copy(vt, ps_v)
                # full 128x128 transposes cover both heads per token-block
                vtp = pp.tile([P, 4, P], BF16, tag="mm", bufs=2, name=f"vtp_{b}_{q4}")
                for t4 in range(4):
                    nc.tensor.matmul(
                        vtp[:, t4],
                        lhsT=vt[:, t4 * P : (t4 + 1) * P],
                        rhs=id_sb,
                        is_transpose=True,
                        start=True, stop=True,
                    )
                nc.vector.tensor_copy(
                    vaug[:, q4 * 4 : (q4 + 1) * 4, :, 0:D],
                    vtp.rearrange("p a (h d) -> p a h d", h=HPC),
                )
            return qt, kt, vaug

        def s_chunks(b, h, qt, kt, at):
            """Generator: S^T causal strips + exp + diagonal mask into at.
            Yields after each chunk so callers can interleave other PE work."""
            hs = h * D
            for c in range(NB):
                w = T - 128 * c
                lhs_k = kt[hs : hs + D, c * P : (c + 1) * P]
                for ck in range(0, w, 512):
                    n = min(512, w - ck)
                    sps = pp.tile([P, 512], F32, tag="wk", bufs=4, name=f"sps_{b}_{h}_{c}_{ck}")
                    nc.tensor.matmul(
                        sps[:, :n], lhsT=lhs_k,
                        rhs=qt[hs : hs + D, 128 * c + ck : 128 * c + ck + n],
                        start=True, stop=True,
                    )
                    o = OFFC[c] + ck
                    nc.scalar.activation(at[:, o : o + n], sps[:, :n], EXP)
                    if ck == 0:  # diagonal block: multiplicative causal mask
                        nc.gpsimd.tensor_tensor(
                            at[:, o : o + P], at[:, o : o + P], mask_sb, MULT
                        )
                    yield

        def avt_phase(b, h, at, vaug, attn_sb, attn1):
            """Weight-stationary transposed AV: attOut^T [65, T] in PSUM
            (row 64 = denominator), then normalize into attn tiles."""
            def finish_drain(g, avp, dsb):
                # rank-1 broadcast of the denominator to partitions 0..63
                rbc = pp.tile([D, 512], F32, tag="rbc", bufs=2, name=f"rbc_{b}_{h}_{g}")
                nc.tensor.matmul(
                    rbc,
                    lhsT=ones_c[D : D + 1, :],
                    rhs=dsb[D : D + 1, :],
                    start=True, stop=True,
                )
                rbc_sb = smallp.tile([D, 512], F32, tag="rbc_sb", bufs=2, name=f"rbcs_{b}_{h}_{g}")
                nc.vector.reciprocal_approx_fast(out=rbc_sb, in_=rbc)
                dst = attn_sb[0:D, g * 512 : (g + 1) * 512] if h == 0 else attn1[:, g * 512 : (g + 1) * 512]
                nc.vector.tensor_tensor(dst, avp[0:D, :], rbc_sb, MULT)
                if h == 1:  # shift head-1 rows to partitions 64..127
                    nc.sync.dma_start(
                        attn_sb[D : 2 * D, g * 512 : (g + 1) * 512],
                        attn1[:, g * 512 : (g + 1) * 512],
                    )

            pend = None
            for g in range(NQ):
                avp = pp.tile([P, 512], F32, tag="wk", bufs=4, name=f"avp_{b}_{h}_{g}")
                for c in range(4 * g + 4):
                    if c <= 4 * g:
                        rhs = at[:, OFFC[c] + 512 * g - 128 * c : OFFC[c] + 512 * g - 128 * c + 512]
                        o1 = 0
                    else:
                        wdt = 512 - 128 * (c - 4 * g)
                        rhs = at[:, OFFC[c] : OFFC[c] + wdt]
                        o1 = 512 - wdt
                    nc.tensor.matmul(
                        avp[0 : D + 1, o1:512],
                        lhsT=vaug[:, c, h],
                        rhs=rhs,
                        start=(c == 0), stop=(c == 4 * g + 3),
                        skip_group_check=True,
                    )
                # denominator row psum->sbuf (stays on partition 64)
                dsb = smallp.tile([D + 1, 512], BF16, tag="dsb", bufs=2, name=f"dsb_{b}_{h}_{g}")
                nc.vector.tensor_copy(dsb[D : D + 1, :], avp[D : D + 1, :])
                if pend is not None:
                    finish_drain(*pend)
                pend = (g, avp, dsb)
            finish_drain(*pend)

        def proj_block(b, attn_sb, j):
            outst = outp.tile([P, C], BF16, tag="outst", name=f"outst_{b}_{j}")
            for n2 in range(2):
                pps = pp.tile([P, 512], F32, tag="mm", bufs=2, name=f"pps_{b}_{j}_{n2}")
                nc.tensor.matmul(
                    pps,
                    lhsT=attn_sb[:, j * P : (j + 1) * P],
                    rhs=wp_bf[:, n2 * 512 : (n2 + 1) * 512],
                    start=True, stop=True,
                )
                nc.vector.tensor_copy(outst[:, n2 * 512 : (n2 + 1) * 512], pps)
            nc.sync.dma_start(out[b * T + j * P : b * T + (j + 1) * P, :], outst)

        # ---- software pipeline over batches ----
        qt, kt, vaug = qkv_phase(0)
        ats = [atp.tile([P, AT_W], BF16, tag="at", name=f"at_0_{h}") for h in range(HPC)]
        for h in range(HPC):
            for _ in s_chunks(0, h, qt, kt, ats[h]):
                pass
        for b in range(B):
            if b + 1 < B:
                qt_n, kt_n, vaug_n = qkv_phase(b + 1)
            attn_sb = attnp.tile([P, T], BF16, tag="attn", name=f"attn_{b}")
            attn1 = attnp.tile([D, T], BF16, tag="attn1", name=f"attn1_{b}")
            for h in range(HPC):
                avt_phase(b, h, ats[h], vaug, attn_sb, attn1)
            js = list(range(NB))
            if b + 1 < B:
                # interleave proj(b) blocks into the exp-bound S(b+1) phase
                ats = [atp.tile([P, AT_W], BF16, tag="at", name=f"at_{b+1}_{h}") for h in range(HPC)]
                nchunk = 0
                for h in range(HPC):
                    for _ in s_chunks(b + 1, h, qt_n, kt_n, ats[h]):
                        nchunk += 1
                        if nchunk >= 16 and (nchunk - 16) % 4 == 0 and js:
                            proj_block(b, attn_sb, js.pop(0))
                qt, kt, vaug = qt_n, kt_n, vaug_n
            for j in js:
                proj_block(b, attn_sb, j)


def build_nc():
    nc = bacc.Bacc("TRN2", debug=False, enable_asserts=False, num_devices=NCORES)
    ins = {
        "xt": nc.dram_tensor("xt", [C, B * T], BF16, kind="ExternalInput").ap(),
        "wq": nc.dram_tensor("wq", [C, P], BF16, kind="ExternalInput").ap(),
        "wk": nc.dram_tensor("wk", [C, P], BF16, kind="ExternalInput").ap(),
        "wv": nc.dram_tensor("wv", [C, P], BF16, kind="ExternalInput").ap(),
        "wp": nc.dram_tensor("wp", [P, C], BF16, kind="ExternalInput").ap(),
        "bq": nc.dram_tensor("bq", [P, 1], F32, kind="ExternalInput").ap(),
        "bk": nc.dram_tensor("bk", [P, 1], F32, kind="ExternalInput").ap(),
        "maskt": nc.dram_tensor("maskt", [P, P], BF16, kind="ExternalInput").ap(),
        "ident": nc.dram_tensor("ident", [P, P], BF16, kind="ExternalInput").ap(),
    }
    outs = {"out": nc.dram_tensor("out", [B * T, C], BF16, kind="ExternalOutput").ap()}
    with tile.TileContext(nc) as tc:
        attention_body(tc, outs, ins)
    nc.compile()
    return nc


def make_in_maps(inputs, W_qkv, b_qkv, W_proj):
    x2 = np.asarray(inputs, np.float32).reshape(B * T, C)
    xtv = np.ascontiguousarray(x2.T).astype(ml_dtypes.bfloat16)
    W_qkv = np.asarray(W_qkv, np.float32)
    b_qkv = np.asarray(b_qkv, np.float32)
    W_proj = np.asarray(W_proj, np.float32)
    identv = np.eye(P, dtype=ml_dtypes.bfloat16)
    masktv = np.triu(np.ones((P, P), np.float32)).astype(ml_dtypes.bfloat16)
    in_maps = []
    for cid in range(NCORES):
        s = cid * HPC * D
        in_maps.append({
            "xt": xtv,
            "wq": np.ascontiguousarray(W_qkv[:, s : s + P]).astype(ml_dtypes.bfloat16),
            "wk": np.ascontiguousarray(W_qkv[:, C + s : C + s + P]).astype(ml_dtypes.bfloat16),
            "wv": np.ascontiguousarray(W_qkv[:, 2 * C + s : 2 * C + s + P]).astype(ml_dtypes.bfloat16),
            "wp": np.ascontiguousarray(W_proj[s : s + P, :]).astype(ml_dtypes.bfloat16),
            "bq": np.ascontiguousarray(b_qkv[s : s + P].reshape(P, 1) * 0.125),
            "bk": np.ascontiguousarray(b_qkv[C + s : C + s + P].reshape(P, 1)),
            "maskt": masktv,
            "ident": identv,
        })
    return in_maps


_NC_CACHE = {}


def run(inputs, W_qkv, b_qkv, W_proj, b_proj, trace=False, **kw):
    from concourse.bass_utils import run_bass_kernel_spmd

    if "nc" not in _NC_CACHE:
        _NC_CACHE["nc"] = build_nc()
    nc = _NC_CACHE["nc"]
    in_maps = make_in_maps(inputs, W_qkv, b_qkv, W_proj)
    res = run_bass_kernel_spmd(nc, in_maps, core_ids=list(range(NCORES)), trace=trace, **kw)
    acc = res.results[0]["out"].astype(np.float32)
    for cid in range(1, NCORES):
        acc += res.results[cid]["out"].astype(np.float32)
    host_bias = np.asarray(b_qkv, np.float32)[2 * C :] @ np.asarray(W_proj, np.float32)
    host_bias = host_bias + np.asarray(b_proj, np.float32)
    outv = (acc + host_bias[None, :]).reshape(B, T, C).astype(np.float32)
    return outv, res


def kernel(inputs, W_qkv, b_qkv, W_proj, b_proj):
    outv, _ = run(inputs, W_qkv, b_qkv, W_proj, b_proj, trace=False)
    return outv
